# revision 12
# baseline (speedup 1.0000x reference)
"""Trainium2 Bass kernel for causal multi-head attention with ALiBi.

Computes, for x:[B,S,D]:
    qkv = x @ W_packed.T + b_packed ; q,k,v = split(qkv)
    heads -> scores = q k^T / sqrt(hd) + alibi_causal_bias
    out = softmax(scores) v -> merge heads -> out @ W_out.T + b_out

Sharding (8 cores): core c handles batch c//4 and heads {k, k+4, k+8, k+12}
(k = c%4), one head per "slot". Host sums the 4 out-projection partials per
batch and adds b_out + W_out @ b_v.

Perf structure: every matmul in the kernel runs in the PE array's (64,128)
row-tiled mode (contraction split into two 64-row halves on tiles T0/T8),
so the array never re-configures (mode switches drain the PE).  128-deep
contractions (QKV, out-proj, attn@v) accumulate the two halves into a PSUM
bank pair merged by one DVE op.  Scores (contraction hd=64) use one tile;
slots with q/k in partitions 0-63 run on T0 while 64-127 slots run on T8,
so two heads' score streams overlap.  Softmax row sums ride a ones-column
appended to v; 1/den is broadcast across partitions by a one-hot f32r
matmul in the same (64,128) mode.  QKV and out-proj are emitted as
single-bank-pair chunks used as PE filler between attention groups (whose
pace is set by the scalar engine's exp).
"""

import os
import sys

import numpy as np

for _p in ("/opt/trn_rl_repo",):
    if os.path.isdir(_p) and _p not in sys.path:
        sys.path.append(_p)

import concourse.bacc as bacc
import concourse.bass as bass
import concourse.tile as tile
from concourse import mybir
from concourse.bass_utils import run_bass_kernel_spmd

B, S, D, H, HD = 2, 2048, 1024, 16, 64
NBLK = S // 128          # 16 k/q blocks
NCORES = 8

F32 = mybir.dt.float32
F32R = mybir.dt.float32r
BF16 = mybir.dt.bfloat16
ADD = mybir.AluOpType.add

# Slots A..D: per-core heads [12+k, 8+k, 4+k, k].  KEEP = causal k-blocks
# kept per q-block (window d_h = 35/slope_h, slot max).  W = q-group width.
SLOT_KEEP = (17, 17, 6, 3)
SLOT_W = (512, 512, 512, 128)
SLOT_OFF0 = (128, 128, 128, 64)
SLOT_TABW = tuple(k + 3 if w == 512 else k for k, w in zip(SLOT_KEEP, SLOT_W))
SLOT_TABOFF = tuple(int(np.cumsum((0,) + SLOT_TABW)[i]) for i in range(4))
TABW = int(sum(SLOT_TABW))  # 52


def _slot_schedule(s):
    """Yield (g, q0, W, [(j, lo, hi, tabcol, isdiag), ...]) per q-group."""
    K, W, _ = SLOT_KEEP[s], SLOT_W[s], SLOT_OFF0[s]
    out = []
    if W == 512:
        for g in range(S // 512):
            jlo = max(0, 4 * g + 3 - (K - 1))
            blocks = []
            for j in range(jlo, 4 * g + 4):
                lo = max(0, (j - 4 * g) * 128)
                hi = min(512, (j - 4 * g + K) * 128)
                m = j - 4 * g + (K - 1)
                blocks.append((j, lo, hi, SLOT_TABOFF[s] + m, j >= 4 * g))
            out.append((g, g * 512, 512, blocks))
    else:
        for i in range(NBLK):
            blocks = []
            for j in range(max(0, i - (K - 1)), i + 1):
                m = j - i + (K - 1)
                blocks.append((j, 0, 128, SLOT_TABOFF[s] + m, j == i))
            out.append((i, i * 128, 128, blocks))
    return out


def build_program():
    nc = bacc.Bacc("TRN2", target_bir_lowering=False, debug=False,
                   num_devices=NCORES)

    xT = nc.dram_tensor("xT", [D, S], BF16, kind="ExternalInput")
    wqkT = nc.dram_tensor("wqkT", [D, 512], BF16, kind="ExternalInput")
    wvT = nc.dram_tensor("wvT", [D, 256], BF16, kind="ExternalInput")
    woT = nc.dram_tensor("woT", [256, D], BF16, kind="ExternalInput")
    bqk = nc.dram_tensor("bqk", [128, 4], F32, kind="ExternalInput")
    btab = nc.dram_tensor("btab", [128, TABW], F32, kind="ExternalInput")
    onehot = nc.dram_tensor("onehot", [128, 128], F32R, kind="ExternalInput")
    zr = nc.dram_tensor("zr", [128, 512], F32R, kind="ExternalInput")
    out = nc.dram_tensor("out", [S, D], BF16, kind="ExternalOutput")
    KDBG = bool(os.environ.get("KDBG"))
    if KDBG:
        d_qk = nc.dram_tensor("d_qk", [128, S], BF16, kind="ExternalOutput")
        d_kt = nc.dram_tensor("d_kt", [128, S], BF16, kind="ExternalOutput")
        d_v = nc.dram_tensor("d_v", [128, 4 * NBLK * 65], BF16,
                             kind="ExternalOutput")
        d_ho = nc.dram_tensor("d_ho", [128, S], BF16, kind="ExternalOutput")
        d_lr = nc.dram_tensor("d_lr", [128, 1024], F32, kind="ExternalOutput")
        d_avs = nc.dram_tensor("d_avs", [65, 512], F32, kind="ExternalOutput")
        d_avq = nc.dram_tensor("d_avq", [65, 512], F32, kind="ExternalOutput")
        d_et = nc.dram_tensor("d_et", [128, 512], BF16, kind="ExternalOutput")

    with tile.TileContext(nc) as tc:
        with tc.tile_pool(name="persist", bufs=1) as pp:
            qkT = [pp.tile([128, S], BF16, tag=f"qkT{t}", name=f"qkT{t}")
                   for t in range(4)]
            v_t = pp.tile([128, 4, NBLK, 65], BF16, tag="v", name="v")
            hoT = [pp.tile([128, S], BF16, tag=f"hoT{t}", name=f"hoT{t}")
                   for t in range(2)]
            btab_sb = pp.tile([128, TABW], F32, tag="btab", name="btab")
            bqk_sb = pp.tile([128, 4], F32, tag="bqk", name="bqk")
            oh_sb = pp.tile([128, 128], F32R, tag="oh", name="oh")
            # den / 1-per-q reciprocal staging rows (row 64 used; rows
            # 65-127 must stay zero: they are contraction rows of the
            # one-hot matmul and garbage there would poison the output)
            lr_den = pp.tile([128, 512], F32R, tag="lrd", name="lrd")

            nc.sync.dma_start(btab_sb[:], btab[:])
            nc.sync.dma_start(bqk_sb[:], bqk[:])
            nc.sync.dma_start(oh_sb[:], onehot[:])
            nc.gpsimd.memset(v_t[:, :, :, 64:65], 1.0)
            nc.sync.dma_start(lr_den[:], zr[:])

            wo_sb = []
            for cc in range(2):
                t = pp.tile([128, D], BF16, tag=f"wo{cc}", name=f"wo{cc}")
                nc.sync.dma_start(t[:], woT[cc * 128:(cc + 1) * 128, :])
                wo_sb.append(t)

            with (
                tc.tile_pool(name="xw", bufs=1) as xw,
                tc.tile_pool(name="et", bufs=8) as etp,
                tc.tile_pool(name="nrm", bufs=3) as nrm,
                tc.tile_pool(name="ob", bufs=2) as obp,
                tc.tile_pool(name="ps_sc", bufs=2, space="PSUM") as sc_ps,
                tc.tile_pool(name="ps_av", bufs=2, space="PSUM") as av_ps,
                tc.tile_pool(name="ps_fa", bufs=2, space="PSUM") as fa_ps,
                tc.tile_pool(name="ps_fb", bufs=2, space="PSUM") as fb_ps,
            ):
                xT_sb, wqk_sb, wv_sb = [], [], []
                for m in range(8):
                    t = xw.tile([128, 512], BF16, tag=f"wqk{m}",
                                name=f"wqk{m}")
                    nc.sync.dma_start(t[:], wqkT[m * 128:(m + 1) * 128, :])
                    wqk_sb.append(t)
                    t = xw.tile([128, S], BF16, tag=f"x{m}", name=f"x{m}")
                    nc.sync.dma_start(t[:], xT[m * 128:(m + 1) * 128, :])
                    xT_sb.append(t)
                    t = xw.tile([128, 256], BF16, tag=f"wv{m}", name=f"wv{m}")
                    nc.sync.dma_start(t[:], wvT[m * 128:(m + 1) * 128, :])
                    wv_sb.append(t)

                # ---- QKV / V / out-proj emitted as bank-pair chunks ----
                def qk_chunk(ft, q4, pool, ptag):
                    scol = slice(q4 * 512, (q4 + 1) * 512)
                    fcol = slice(ft * 128, (ft + 1) * 128)
                    ps = pool.tile([128, 512], F32, tag=ptag, name="qkps")
                    for m in range(8):
                        nc.tensor.matmul(
                            ps[:], wqk_sb[m][:, fcol], xT_sb[m][:, scol],
                            start=(m == 0), stop=(m == 7))
                    # psum*scale + bias (1/sqrt(hd) folded into q side)
                    nc.vector.tensor_scalar(
                        out=qkT[ft][:, scol], in0=ps[:],
                        scalar1=(0.125 if ft < 2 else 1.0),
                        scalar2=bqk_sb[:, ft:ft + 1],
                        op0=mybir.AluOpType.mult,
                        op1=mybir.AluOpType.add,
                    )

                def v_chunk(sb, pool, ptag):
                    scol = slice(sb * 128, (sb + 1) * 128)
                    ps = pool.tile([128, 256], F32, tag=ptag, name="vps")
                    for m in range(8):
                        nc.tensor.matmul(
                            ps[:], xT_sb[m][:, scol], wv_sb[m][:],
                            start=(m == 0), stop=(m == 7))
                    nc.vector.tensor_copy(
                        v_t[:, :, sb, 0:64],
                        ps[:].rearrange("p (s c) -> p s c", s=4))

                def op_block(sb, pool, ptag):
                    ob = obp.tile([128, D], BF16, tag="ob", name="ob")
                    bcol = slice(sb * 128, (sb + 1) * 128)
                    for jh in range(2):
                        jcol = slice(jh * 512, (jh + 1) * 512)
                        ps = pool.tile([128, 512], F32, tag=ptag, name="opps")
                        for cc in range(2):
                            nc.tensor.matmul(
                                ps[:], hoT[cc][:, bcol], wo_sb[cc][:, jcol],
                                start=(cc == 0), stop=(cc == 1))
                        nc.vector.tensor_copy(ob[:, jcol], ps[:])
                    nc.gpsimd.dma_start(out[bcol, :], ob[:])

                # ---- attention ----
                sched = [_slot_schedule(s) for s in range(4)]

                def attn_group(s, ent):
                    g, q0, W, blocks = ent
                    po = (s % 2) * 64
                    qT_s = qkT[s // 2][po:po + 64, :]
                    kT_s = qkT[2 + s // 2][po:po + 64, :]
                    nb = len(blocks)
                    avP = av_ps.tile([65, W], F32, tag="av", name="avP")
                    avQ = av_ps.tile([65, W], F32, tag="av", name="avQ")
                    for bi, (j, lo, hi, tcol, isdiag) in enumerate(blocks):
                        sc = sc_ps.tile([128, W], F32, tag="sc", name="sc")
                        nc.tensor.matmul(
                            sc[:], kT_s[:, j * 128:(j + 1) * 128],
                            qT_s[:, q0:q0 + W])
                        et = etp.tile([128, W], BF16, tag="et", name="et")
                        if lo > 0:
                            nc.gpsimd.memset(et[:, 0:lo], 0.0)
                        if hi < W:
                            nc.gpsimd.memset(et[:, hi:W], 0.0)
                        nc.scalar.activation(
                            et[:, lo:hi], sc[:, lo:hi],
                            mybir.ActivationFunctionType.Exp,
                            bias=btab_sb[:, tcol:tcol + 1], scale=1.0)
                        if isdiag:
                            # zero k>q inside the diagonal 128x128 block
                            nc.gpsimd.affine_select(
                                out=et[:, lo:lo + 128],
                                in_=et[:, lo:lo + 128],
                                compare_op=mybir.AluOpType.is_ge,
                                fill=0.0, base=0,
                                pattern=[[1, 128]],
                                channel_multiplier=-1,
                            )
                        if KDBG and s == 3 and g == 15 and isdiag:
                            nc.gpsimd.dma_start(d_et[:, 0:W], et[:])
                        nc.tensor.matmul(
                            avP[:], v_t[0:64, s, j, :], et[0:64, :],
                            start=(bi == 0), stop=(bi == nb - 1))
                        nc.tensor.matmul(
                            avQ[:], v_t[64:128, s, j, :], et[64:128, :],
                            start=(bi == 0), stop=(bi == nb - 1))
                    # softmax denominators: den row rides as v column 64.
                    # DVE reads at most one PSUM operand: stage avP first.
                    avs = nrm.tile([65, W], F32, tag="avs", name="avs")
                    nc.vector.tensor_copy(avs[:], avP[:])
                    if KDBG and s == 3 and g == 15:
                        nc.gpsimd.dma_start(d_avs[:, 0:W], avs[:])
                        avqs = nrm.tile([65, W], F32, tag="avs", name="avqs")
                        nc.vector.tensor_copy(avqs[:], avQ[:])
                        nc.gpsimd.dma_start(d_avq[:, 0:W], avqs[:])
                    nc.vector.tensor_add(
                        lr_den[64:65, 0:W], avs[64:65, :], avQ[64:65, :])
                    avm = nrm.tile([64, W], F32, tag="avm", name="avm")
                    nc.vector.tensor_add(avm[:], avs[0:64, :], avQ[0:64, :])
                    bc = av_ps.tile([128, W], F32, tag="av", name="bc")
                    nc.tensor.matmul(
                        bc[:], oh_sb[64:128, :], lr_den[64:128, 0:W])
                    binv = nrm.tile([64, W], F32, tag="binv", name="binv")
                    nc.vector.reciprocal_approx_fast(
                        out=binv[:], in_=bc[0:64, :])
                    hoT_s = hoT[s // 2]
                    if po == 0:
                        nc.vector.tensor_mul(
                            hoT_s[0:64, q0:q0 + W], avm[:], binv[:])
                    else:
                        # DVE lanes can't shift partitions; bounce via DMA
                        tmp = nrm.tile([64, W], BF16, tag="hotmp",
                                       name="hotmp")
                        nc.vector.tensor_mul(tmp[:], avm[:], binv[:])
                        nc.gpsimd.dma_start(
                            hoT_s[64:128, q0:q0 + W], tmp[:])

                # ---- emission ----
                # startup: qkv chunks for q half 0, 3 bank-pairs deep
                rot = [(fa_ps, "fa"), (fb_ps, "fb"), (av_ps, "av")]
                ri = 0

                def next_pool():
                    nonlocal ri
                    p = rot[ri % len(rot)]
                    ri += 1
                    return p

                for q4 in range(2):
                    for ft in range(4):
                        p, t = next_pool()
                        qk_chunk(ft, q4, p, t)
                    for sb in (4 * q4, 4 * q4 + 1, 4 * q4 + 2, 4 * q4 + 3):
                        p, t = next_pool()
                        v_chunk(sb, p, t)

                # filler queue consumed between attention groups
                fillers = []
                for q4 in (2, 3):
                    for ft in range(4):
                        fillers.append(("qk", ft, q4))
                for sb in range(8, 16):
                    fillers.append(("v", sb))
                frot = [(fa_ps, "fa"), (fb_ps, "fb")]
                fi = 0

                def pop_fillers(n):
                    nonlocal fi
                    for _ in range(n):
                        if not fillers:
                            return
                        kind, *a = fillers.pop(0)
                        p, t = frot[fi % 2]
                        fi += 1
                        if kind == "qk":
                            qk_chunk(a[0], a[1], p, t)
                        elif kind == "v":
                            v_chunk(a[0], p, t)
                        else:
                            op_block(a[0], p, t)

                for g in range(4):
                    for s in range(3):
                        attn_group(s, sched[s][g])
                        pop_fillers(2)
                    for i4 in range(4):
                        attn_group(3, sched[3][4 * g + i4])
                    pop_fillers(2)
                    for sb in range(4 * g, 4 * g + 4):
                        fillers.append(("op", sb))
                while fillers:
                    pop_fillers(1)

                if KDBG:
                    nc.gpsimd.dma_start(d_qk[:], qkT[0][:])
                    nc.gpsimd.dma_start(d_kt[:], qkT[2][:])
                    nc.gpsimd.dma_start(
                        d_v[:], v_t[:].rearrange("p a b c -> p (a b c)"))
                    nc.gpsimd.dma_start(d_ho[:], hoT[0][:])
                    nc.gpsimd.dma_start(d_lr[:, 0:512], lr_den[:])

    nc.compile()
    return nc


def make_core_inputs(c, x, W_packed, b_packed):
    """Host-side shard prep for core c (pure numpy reshuffles)."""
    k, b = c % 4, c // 4
    heads = [12 + k, 8 + k, 4 + k, k]          # slots A..D
    rows = np.concatenate([np.arange(h * 64, (h + 1) * 64) for h in heads])

    xT = np.ascontiguousarray(x[b].T)                       # [D, S]
    wq = W_packed[rows]
    wk = W_packed[D + rows]
    wv = W_packed[2 * D + rows]
    wqkT = np.ascontiguousarray(np.concatenate([wq, wk], 0).T)  # [D, 512]
    wvT = np.ascontiguousarray(wv.T)                        # [D, 256]

    bq = b_packed[rows] / 8.0
    bk = b_packed[D + rows]
    bqk = np.stack([bq[:128], bq[128:], bk[:128], bk[128:]], 1)  # [128, 4]
    bqk = np.ascontiguousarray(bqk, dtype=np.float32)

    btab = np.zeros((128, TABW), np.float32)
    p = np.arange(128, dtype=np.float64)[:, None]
    for s in range(4):
        h = heads[s]
        slope = 2.0 ** (-(h + 1) * 8.0 / H)
        K, off0, tw, to = SLOT_KEEP[s], SLOT_OFF0[s], SLOT_TABW[s], SLOT_TABOFF[s]
        m = np.arange(tw, dtype=np.float64)[None, :]
        btab[:, to:to + tw] = (slope * (p + 128.0 * (m - (K - 1)) - off0)
                               ).astype(np.float32)
    import ml_dtypes
    onehot = np.zeros((128, 128), np.float32)
    onehot[64, 0:64] = 1.0
    return heads, {"xT": xT.astype(ml_dtypes.bfloat16),
                   "wqkT": wqkT.astype(ml_dtypes.bfloat16),
                   "wvT": wvT.astype(ml_dtypes.bfloat16),
                   "bqk": bqk, "btab": btab, "onehot": onehot,
                   "zr": np.zeros((128, 512), np.float32)}


_NC_CACHE = {}


def _get_program():
    if "nc" not in _NC_CACHE:
        _NC_CACHE["nc"] = build_program()
    return _NC_CACHE["nc"]


def kernel(x, W_packed, b_packed, W_out, b_out):
    x = np.asarray(x, np.float32)
    W_packed = np.asarray(W_packed, np.float32)
    b_packed = np.asarray(b_packed, np.float32)
    W_out = np.asarray(W_out, np.float32)
    b_out = np.asarray(b_out, np.float32)

    nc = _get_program()

    in_maps = []
    for c in range(NCORES):
        heads, m = make_core_inputs(c, x, W_packed, b_packed)
        cols = np.concatenate([np.arange(h * 64, (h + 1) * 64) for h in heads])
        import ml_dtypes
        m["woT"] = np.ascontiguousarray(W_out[:, cols].T).astype(
            ml_dtypes.bfloat16)
        in_maps.append(m)

    res = run_bass_kernel_spmd(nc, in_maps, core_ids=list(range(NCORES)))

    # Gather: sum partials per batch; add b_out and the folded v-bias term.
    b_v = b_packed[2 * D:]
    bias_row = (b_out + W_out @ b_v).astype(np.float32)     # [D]
    full = np.empty((B, S, D), np.float32)
    for b in range(B):
        acc = res.results[4 * b]["out"].astype(np.float32).copy()
        for c in range(4 * b + 1, 4 * b + 4):
            acc += res.results[c]["out"]
        full[b] = acc + bias_row
    return full


# revision 13
# speedup vs baseline: 1.0361x; 1.0361x over previous
"""Trainium2 Bass kernel for causal multi-head attention with ALiBi.

Computes, for x:[B,S,D]:
    qkv = x @ W_packed.T + b_packed ; q,k,v = split(qkv)
    heads -> scores = q k^T / sqrt(hd) + alibi_causal_bias
    out = softmax(scores) v -> merge heads -> out @ W_out.T + b_out

Sharding (8 cores): core c handles batch c//4 and heads {k, k+4, k+8, k+12}
(k = c%4), one head per "slot". Host sums the 4 out-projection partials per
batch and adds b_out + W_out @ b_v.

Perf structure: every matmul runs the PE array in the same (128,128)
configuration - reconfiguring the array's tiling mode drains it (~110ns,
measured), and a per-head schedule would pay that on every score/AV matmul
pair.  Scores contract over hd=64, so each slot's K tile is stored
zero-padded to 128 partitions (data on its own half, zeros on the other);
the packed Q tile streams both heads and the zero half of K kills the
other head's contribution.  Softmax row sums ride a ones-column appended
to v; 1/den is broadcast across partitions by a one-hot f32r matmul.
QKV and out-proj are emitted as single-bank chunks woven between
attention groups as PE filler under the scalar engine's exp stream.

ALiBi sparsity: head h attends effectively only ~35/slope_h positions
back. Slots keep only the causal k-blocks within that window.  Softmax
without row-max: exp is recentred per q-group by a per-column constant
that cancels in normalization (rides the Exp bias table).
"""

import os
import sys

import numpy as np

for _p in ("/opt/trn_rl_repo",):
    if os.path.isdir(_p) and _p not in sys.path:
        sys.path.append(_p)

import concourse.bacc as bacc
import concourse.bass as bass
import concourse.tile as tile
from concourse import mybir
from concourse.bass_utils import run_bass_kernel_spmd

B, S, D, H, HD = 2, 2048, 1024, 16, 64
NBLK = S // 128          # 16 k/q blocks
NCORES = 8

F32 = mybir.dt.float32
F32R = mybir.dt.float32r
BF16 = mybir.dt.bfloat16

SLOT_KEEP = (17, 17, 6, 3)
SLOT_W = (512, 512, 512, 128)
SLOT_OFF0 = (128, 128, 128, 64)
SLOT_TABW = tuple(k + 3 if w == 512 else k for k, w in zip(SLOT_KEEP, SLOT_W))
SLOT_TABOFF = tuple(int(np.cumsum((0,) + SLOT_TABW)[i]) for i in range(4))
TABW = int(sum(SLOT_TABW))  # 52


def _slot_schedule(s):
    """Yield (g, q0, W, [(j, lo, hi, tabcol, isdiag), ...]) per q-group."""
    K, W, _ = SLOT_KEEP[s], SLOT_W[s], SLOT_OFF0[s]
    out = []
    if W == 512:
        for g in range(S // 512):
            jlo = max(0, 4 * g + 3 - (K - 1))
            blocks = []
            for j in range(jlo, 4 * g + 4):
                lo = max(0, (j - 4 * g) * 128)
                hi = min(512, (j - 4 * g + K) * 128)
                m = j - 4 * g + (K - 1)
                blocks.append((j, lo, hi, SLOT_TABOFF[s] + m, j >= 4 * g))
            out.append((g, g * 512, 512, blocks))
    else:
        for i in range(NBLK):
            blocks = []
            for j in range(max(0, i - (K - 1)), i + 1):
                m = j - i + (K - 1)
                blocks.append((j, 0, 128, SLOT_TABOFF[s] + m, j == i))
            out.append((i, i * 128, 128, blocks))
    return out


def build_program():
    nc = bacc.Bacc("TRN2", target_bir_lowering=False, debug=False,
                   num_devices=NCORES)

    xT = nc.dram_tensor("xT", [D, S], BF16, kind="ExternalInput")
    wqkT = nc.dram_tensor("wqkT", [D, 512], BF16, kind="ExternalInput")
    wvT = nc.dram_tensor("wvT", [D, 256], BF16, kind="ExternalInput")
    woT = nc.dram_tensor("woT", [256, D], BF16, kind="ExternalInput")
    bqk = nc.dram_tensor("bqk", [128, 4], F32, kind="ExternalInput")
    btab = nc.dram_tensor("btab", [128, TABW], F32, kind="ExternalInput")
    onehot = nc.dram_tensor("onehot", [128, 128], F32R, kind="ExternalInput")
    zr = nc.dram_tensor("zr", [128, 512], F32R, kind="ExternalInput")
    out = nc.dram_tensor("out", [S, D], BF16, kind="ExternalOutput")

    with tile.TileContext(nc) as tc:
        with tc.tile_pool(name="persist", bufs=1) as pp:
            # q tiles packed (2 heads per tile); k tiles zero-padded per slot
            qT = [pp.tile([128, S], BF16, tag=f"qT{t}", name=f"qT{t}")
                  for t in range(2)]
            kpk = [pp.tile([128, S], BF16, tag=f"kpk{t}", name=f"kpk{t}")
                   for t in range(2)]
            kT = [pp.tile([128, S], BF16, tag=f"kT{s}", name=f"kT{s}")
                  for s in range(4)]
            v_t = pp.tile([128, 4, NBLK, 65], BF16, tag="v", name="v")
            hoT = [pp.tile([128, S], BF16, tag=f"hoT{t}", name=f"hoT{t}")
                   for t in range(2)]
            btab_sb = pp.tile([128, TABW], F32, tag="btab", name="btab")
            bqk_sb = pp.tile([128, 4], F32, tag="bqk", name="bqk")
            oh_sb = pp.tile([128, 128], F32R, tag="oh", name="oh")
            # den staging row (row 64); other rows must stay zero (they are
            # contraction rows of the one-hot matmul; garbage would poison)
            lr_den = pp.tile([128, 512], F32R, tag="lrd", name="lrd")

            nc.sync.dma_start(btab_sb[:], btab[:])
            nc.sync.dma_start(bqk_sb[:], bqk[:])
            nc.sync.dma_start(oh_sb[:], onehot[:])
            nc.sync.dma_start(lr_den[:], zr[:])
            nc.gpsimd.memset(v_t[:, :, :, 64:65], 1.0)
            # zero the pad half of each slot's K tile (once)
            for s in range(4):
                po = (s % 2) * 64
                nc.gpsimd.memset(kT[s][64 - po:128 - po, :], 0.0)

            wo_sb = []
            for cc in range(2):
                t = pp.tile([128, D], BF16, tag=f"wo{cc}", name=f"wo{cc}")
                nc.sync.dma_start(t[:], woT[cc * 128:(cc + 1) * 128, :])
                wo_sb.append(t)

            with (
                tc.tile_pool(name="xw", bufs=1) as xw,
                tc.tile_pool(name="et", bufs=8) as etp,
                tc.tile_pool(name="nrm", bufs=3) as nrm,
                tc.tile_pool(name="ob", bufs=2) as obp,
                tc.tile_pool(name="ps_sc", bufs=3, space="PSUM") as sc_ps,
                tc.tile_pool(name="ps_av", bufs=2, space="PSUM") as av_ps,
                tc.tile_pool(name="ps_fa", bufs=2, space="PSUM") as fa_ps,
                tc.tile_pool(name="ps_fb", bufs=1, space="PSUM") as fb_ps,
            ):
                xT_sb, wqk_sb, wv_sb = [], [], []
                for m in range(8):
                    t = xw.tile([128, 512], BF16, tag=f"wqk{m}",
                                name=f"wqk{m}")
                    nc.sync.dma_start(t[:], wqkT[m * 128:(m + 1) * 128, :])
                    wqk_sb.append(t)
                    t = xw.tile([128, S], BF16, tag=f"x{m}", name=f"x{m}")
                    nc.sync.dma_start(t[:], xT[m * 128:(m + 1) * 128, :])
                    xT_sb.append(t)
                    t = xw.tile([128, 256], BF16, tag=f"wv{m}", name=f"wv{m}")
                    nc.sync.dma_start(t[:], wvT[m * 128:(m + 1) * 128, :])
                    wv_sb.append(t)

                # ---- QKV / V / out-proj single-bank chunks ----
                def qk_chunk(ft, q4, pool, ptag):
                    scol = slice(q4 * 512, (q4 + 1) * 512)
                    fcol = slice(ft * 128, (ft + 1) * 128)
                    ps = pool.tile([128, 512], F32, tag=ptag, name="qkps")
                    for m in range(8):
                        nc.tensor.matmul(
                            ps[:], wqk_sb[m][:, fcol], xT_sb[m][:, scol],
                            start=(m == 0), stop=(m == 7))
                    dst = (qT if ft < 2 else kpk)[ft % 2]
                    nc.vector.tensor_scalar(
                        out=dst[:, scol], in0=ps[:],
                        scalar1=(0.125 if ft < 2 else 1.0),
                        scalar2=bqk_sb[:, ft:ft + 1],
                        op0=mybir.AluOpType.mult,
                        op1=mybir.AluOpType.add,
                    )
                    if ft >= 2:
                        # scatter the two heads into their padded K tiles
                        t = ft - 2
                        nc.gpsimd.tensor_copy(
                            kT[2 * t][0:64, scol], dst[0:64, scol])
                        nc.gpsimd.tensor_copy(
                            kT[2 * t + 1][64:128, scol], dst[64:128, scol])

                def v_chunk(sb, pool, ptag):
                    scol = slice(sb * 128, (sb + 1) * 128)
                    ps = pool.tile([128, 256], F32, tag=ptag, name="vps")
                    for m in range(8):
                        nc.tensor.matmul(
                            ps[:], xT_sb[m][:, scol], wv_sb[m][:],
                            start=(m == 0), stop=(m == 7))
                    nc.vector.tensor_copy(
                        v_t[:, :, sb, 0:64],
                        ps[:].rearrange("p (s c) -> p s c", s=4))

                def op_block(sb, pool, ptag):
                    ob = obp.tile([128, D], BF16, tag="ob", name="ob")
                    bcol = slice(sb * 128, (sb + 1) * 128)
                    for jh in range(2):
                        jcol = slice(jh * 512, (jh + 1) * 512)
                        ps = pool.tile([128, 512], F32, tag=ptag, name="opps")
                        for cc in range(2):
                            nc.tensor.matmul(
                                ps[:], hoT[cc][:, bcol], wo_sb[cc][:, jcol],
                                start=(cc == 0), stop=(cc == 1))
                        nc.vector.tensor_copy(ob[:, jcol], ps[:])
                    nc.gpsimd.dma_start(out[bcol, :], ob[:])

                # ---- attention ----
                sched = [_slot_schedule(s) for s in range(4)]

                def attn_group(s, ent):
                    g, q0, W, blocks = ent
                    qT_s = qT[s // 2]
                    kT_s = kT[s]
                    nb = len(blocks)
                    av = av_ps.tile([65, W], F32, tag="av", name="av")
                    for bi, (j, lo, hi, tcol, isdiag) in enumerate(blocks):
                        sc = sc_ps.tile([128, W], F32, tag="sc", name="sc")
                        nc.tensor.matmul(
                            sc[:], kT_s[:, j * 128:(j + 1) * 128],
                            qT_s[:, q0:q0 + W])
                        et = etp.tile([128, W], BF16, tag="et", name="et")
                        if lo > 0:
                            nc.gpsimd.memset(et[:, 0:lo], 0.0)
                        if hi < W:
                            nc.gpsimd.memset(et[:, hi:W], 0.0)
                        nc.scalar.activation(
                            et[:, lo:hi], sc[:, lo:hi],
                            mybir.ActivationFunctionType.Exp,
                            bias=btab_sb[:, tcol:tcol + 1], scale=1.0)
                        if isdiag:
                            # zero k>q inside the diagonal 128x128 block
                            nc.gpsimd.affine_select(
                                out=et[:, lo:lo + 128],
                                in_=et[:, lo:lo + 128],
                                compare_op=mybir.AluOpType.is_ge,
                                fill=0.0, base=0,
                                pattern=[[1, 128]],
                                channel_multiplier=-1,
                            )
                        nc.tensor.matmul(
                            av[:], v_t[:, s, j, :], et[:],
                            start=(bi == 0), stop=(bi == nb - 1))
                    # softmax denominator: ones-column sum broadcast via
                    # one-hot matmul, then reciprocal + scale
                    nc.vector.tensor_copy(lr_den[64:65, 0:W], av[64:65, :])
                    bc = sc_ps.tile([128, W], F32, tag="sc", name="bc")
                    nc.tensor.matmul(bc[:], oh_sb[:], lr_den[:, 0:W])
                    binv = nrm.tile([64, W], F32, tag="binv", name="binv")
                    nc.vector.reciprocal_approx_fast(
                        out=binv[:], in_=bc[0:64, :])
                    hoT_s = hoT[s // 2]
                    po = (s % 2) * 64
                    if po == 0:
                        nc.vector.tensor_mul(
                            hoT_s[0:64, q0:q0 + W], av[0:64, :], binv[:])
                    else:
                        # DVE lanes can't shift partitions; bounce via DMA
                        tmp = nrm.tile([64, W], BF16, tag="hotmp",
                                       name="hotmp")
                        nc.vector.tensor_mul(tmp[:], av[0:64, :], binv[:])
                        nc.gpsimd.dma_start(
                            hoT_s[64:128, q0:q0 + W], tmp[:])

                # ---- emission ----
                # startup: all chunks for q half 0, deep bank rotation
                rot = [(fa_ps, "fa"), (fb_ps, "fb"), (av_ps, "av"),
                       (sc_ps, "sc")]
                ri = 0

                def next_pool():
                    nonlocal ri
                    p = rot[ri % len(rot)]
                    ri += 1
                    return p

                for q4 in range(2):
                    for ft in (2, 3, 0, 1):       # k first: extra copy hop
                        p, t = next_pool()
                        qk_chunk(ft, q4, p, t)
                    for sb in (4 * q4, 4 * q4 + 1, 4 * q4 + 2, 4 * q4 + 3):
                        p, t = next_pool()
                        v_chunk(sb, p, t)

                # filler queue consumed between attention groups
                fillers = []
                for q4 in (2, 3):
                    for ft in (2, 3, 0, 1):
                        fillers.append(("qk", ft, q4))
                for sb in range(8, 16):
                    fillers.append(("v", sb))
                frot = [(fa_ps, "fa"), (fb_ps, "fb")]
                fi = 0

                def pop_fillers(n):
                    nonlocal fi
                    for _ in range(n):
                        if not fillers:
                            return
                        kind, *a = fillers.pop(0)
                        p, t = frot[fi % 2]
                        fi += 1
                        if kind == "qk":
                            qk_chunk(a[0], a[1], p, t)
                        elif kind == "v":
                            v_chunk(a[0], p, t)
                        else:
                            op_block(a[0], p, t)

                for g in range(4):
                    for s in range(3):
                        attn_group(s, sched[s][g])
                        pop_fillers(2)
                    for i4 in range(4):
                        attn_group(3, sched[3][4 * g + i4])
                    pop_fillers(2)
                    for sb in range(4 * g, 4 * g + 4):
                        fillers.append(("op", sb))
                while fillers:
                    pop_fillers(1)

    nc.compile()
    return nc


def make_core_inputs(c, x, W_packed, b_packed):
    """Host-side shard prep for core c (pure numpy reshuffles)."""
    k, b = c % 4, c // 4
    heads = [12 + k, 8 + k, 4 + k, k]          # slots A..D
    rows = np.concatenate([np.arange(h * 64, (h + 1) * 64) for h in heads])

    xT = np.ascontiguousarray(x[b].T)                       # [D, S]
    wq = W_packed[rows]
    wk = W_packed[D + rows]
    wv = W_packed[2 * D + rows]
    wqkT = np.ascontiguousarray(np.concatenate([wq, wk], 0).T)  # [D, 512]
    wvT = np.ascontiguousarray(wv.T)                        # [D, 256]

    bq = b_packed[rows] / 8.0
    bk = b_packed[D + rows]
    bqk = np.stack([bq[:128], bq[128:], bk[:128], bk[128:]], 1)  # [128, 4]
    bqk = np.ascontiguousarray(bqk, dtype=np.float32)

    btab = np.zeros((128, TABW), np.float32)
    p = np.arange(128, dtype=np.float64)[:, None]
    for s in range(4):
        h = heads[s]
        slope = 2.0 ** (-(h + 1) * 8.0 / H)
        K, off0, tw, to = SLOT_KEEP[s], SLOT_OFF0[s], SLOT_TABW[s], SLOT_TABOFF[s]
        m = np.arange(tw, dtype=np.float64)[None, :]
        btab[:, to:to + tw] = (slope * (p + 128.0 * (m - (K - 1)) - off0)
                               ).astype(np.float32)
    import ml_dtypes
    onehot = np.zeros((128, 128), np.float32)
    onehot[64, 0:64] = 1.0
    return heads, {"xT": xT.astype(ml_dtypes.bfloat16),
                   "wqkT": wqkT.astype(ml_dtypes.bfloat16),
                   "wvT": wvT.astype(ml_dtypes.bfloat16),
                   "bqk": bqk, "btab": btab, "onehot": onehot,
                   "zr": np.zeros((128, 512), np.float32)}


_NC_CACHE = {}


def _get_program():
    if "nc" not in _NC_CACHE:
        _NC_CACHE["nc"] = build_program()
    return _NC_CACHE["nc"]


def kernel(x, W_packed, b_packed, W_out, b_out):
    x = np.asarray(x, np.float32)
    W_packed = np.asarray(W_packed, np.float32)
    b_packed = np.asarray(b_packed, np.float32)
    W_out = np.asarray(W_out, np.float32)
    b_out = np.asarray(b_out, np.float32)

    nc = _get_program()

    in_maps = []
    for c in range(NCORES):
        heads, m = make_core_inputs(c, x, W_packed, b_packed)
        cols = np.concatenate([np.arange(h * 64, (h + 1) * 64) for h in heads])
        import ml_dtypes
        m["woT"] = np.ascontiguousarray(W_out[:, cols].T).astype(
            ml_dtypes.bfloat16)
        in_maps.append(m)

    res = run_bass_kernel_spmd(nc, in_maps, core_ids=list(range(NCORES)))

    # Gather: sum partials per batch; add b_out and the folded v-bias term.
    b_v = b_packed[2 * D:]
    bias_row = (b_out + W_out @ b_v).astype(np.float32)     # [D]
    full = np.empty((B, S, D), np.float32)
    for b in range(B):
        acc = res.results[4 * b]["out"].astype(np.float32).copy()
        for c in range(4 * b + 1, 4 * b + 4):
            acc += res.results[c]["out"]
        full[b] = acc + bias_row
    return full


# revision 14
# speedup vs baseline: 1.1669x; 1.1262x over previous
"""Trainium2 Bass kernel for causal multi-head attention with ALiBi.

Computes, for x:[B,S,D]:
    qkv = x @ W_packed.T + b_packed ; q,k,v = split(qkv)
    heads -> scores = q k^T / sqrt(hd) + alibi_causal_bias
    out = softmax(scores) v -> merge heads -> out @ W_out.T + b_out

Sharding (8 cores): core c handles batch c//4 and heads {k, k+4, k+8, k+12}
(k = c%4), one head per "slot". Host sums the 4 out-projection partials per
batch and adds b_out + W_out @ b_v.

Perf structure: every matmul runs the PE array in the same (128,128)
configuration - reconfiguring the array's tiling mode drains it (~110ns,
measured), and a per-head schedule would pay that on every score/AV matmul
pair.  Scores contract over hd=64, so each slot's K tile is stored
zero-padded to 128 partitions (data on its own half, zeros on the other);
the packed Q tile streams both heads and the zero half of K kills the
other head's contribution.  Softmax row sums ride a ones-column appended
to v; 1/den is broadcast across partitions by a one-hot f32r matmul.
QKV and out-proj are emitted as single-bank chunks woven between
attention groups as PE filler under the scalar engine's exp stream.

ALiBi sparsity: head h attends effectively only ~35/slope_h positions
back. Slots keep only the causal k-blocks within that window.  Softmax
without row-max: exp is recentred per q-group by a per-column constant
that cancels in normalization (rides the Exp bias table).
"""

import os
import sys

import numpy as np

for _p in ("/opt/trn_rl_repo",):
    if os.path.isdir(_p) and _p not in sys.path:
        sys.path.append(_p)

import concourse.bacc as bacc
import concourse.bass as bass
import concourse.tile as tile
from concourse import mybir
from concourse.bass_utils import run_bass_kernel_spmd

B, S, D, H, HD = 2, 2048, 1024, 16, 64
NBLK = S // 128          # 16 k/q blocks
NCORES = 8

F32 = mybir.dt.float32
F32R = mybir.dt.float32r
BF16 = mybir.dt.bfloat16

SLOT_KEEP = (17, 17, 6, 3)
SLOT_W = (512, 512, 512, 128)
SLOT_OFF0 = (128, 128, 128, 64)
SLOT_TABW = tuple(k + 3 if w == 512 else k for k, w in zip(SLOT_KEEP, SLOT_W))
SLOT_TABOFF = tuple(int(np.cumsum((0,) + SLOT_TABW)[i]) for i in range(4))
TABW = int(sum(SLOT_TABW))  # 52


def _slot_schedule(s):
    """Yield (g, q0, W, [(j, lo, hi, tabcol, isdiag), ...]) per q-group."""
    K, W, _ = SLOT_KEEP[s], SLOT_W[s], SLOT_OFF0[s]
    out = []
    if W == 512:
        for g in range(S // 512):
            jlo = max(0, 4 * g + 3 - (K - 1))
            blocks = []
            for j in range(jlo, 4 * g + 4):
                lo = max(0, (j - 4 * g) * 128)
                hi = min(512, (j - 4 * g + K) * 128)
                m = j - 4 * g + (K - 1)
                blocks.append((j, lo, hi, SLOT_TABOFF[s] + m, j >= 4 * g))
            out.append((g, g * 512, 512, blocks))
    else:
        for i in range(NBLK):
            blocks = []
            for j in range(max(0, i - (K - 1)), i + 1):
                m = j - i + (K - 1)
                blocks.append((j, 0, 128, SLOT_TABOFF[s] + m, j == i))
            out.append((i, i * 128, 128, blocks))
    return out


def build_program():
    nc = bacc.Bacc("TRN2", target_bir_lowering=False, debug=False,
                   num_devices=NCORES)

    xT = nc.dram_tensor("xT", [D, S], BF16, kind="ExternalInput")
    wqkT = nc.dram_tensor("wqkT", [D, 512], BF16, kind="ExternalInput")
    wvT = nc.dram_tensor("wvT", [D, 256], BF16, kind="ExternalInput")
    woT = nc.dram_tensor("woT", [256, D], BF16, kind="ExternalInput")
    bqk = nc.dram_tensor("bqk", [128, 4], F32, kind="ExternalInput")
    btab = nc.dram_tensor("btab", [128, TABW], F32, kind="ExternalInput")
    onehot = nc.dram_tensor("onehot", [128, 128], F32R, kind="ExternalInput")
    zr = nc.dram_tensor("zr", [128, 512], F32R, kind="ExternalInput")
    out = nc.dram_tensor("out", [S, D], BF16, kind="ExternalOutput")

    with tile.TileContext(nc) as tc:
        with tc.tile_pool(name="persist", bufs=1) as pp:
            # q tiles packed (2 heads per tile); k tiles zero-padded per slot
            qT = [pp.tile([128, S], BF16, tag=f"qT{t}", name=f"qT{t}")
                  for t in range(2)]
            kpk = [pp.tile([128, S], BF16, tag=f"kpk{t}", name=f"kpk{t}")
                   for t in range(2)]
            kT = [pp.tile([128, S], BF16, tag=f"kT{s}", name=f"kT{s}")
                  for s in range(4)]
            v_t = pp.tile([128, 4, NBLK, 65], BF16, tag="v", name="v")
            hoT = [pp.tile([128, S], BF16, tag=f"hoT{t}", name=f"hoT{t}")
                   for t in range(2)]
            btab_sb = pp.tile([128, TABW], F32, tag="btab", name="btab")
            bqk_sb = pp.tile([128, 4], F32, tag="bqk", name="bqk")
            oh_sb = pp.tile([128, 128], F32R, tag="oh", name="oh")
            # den staging row (row 64); other rows must stay zero (they are
            # contraction rows of the one-hot matmul; garbage would poison)
            lr_den = pp.tile([128, 512], F32R, tag="lrd", name="lrd")

            nc.sync.dma_start(btab_sb[:], btab[:])
            nc.sync.dma_start(bqk_sb[:], bqk[:])
            nc.sync.dma_start(oh_sb[:], onehot[:])
            nc.sync.dma_start(lr_den[:], zr[:])
            nc.gpsimd.memset(v_t[:, :, :, 64:65], 1.0)
            # zero the pad half of each slot's K tile (once)
            for s in range(4):
                po = (s % 2) * 64
                nc.gpsimd.memset(kT[s][64 - po:128 - po, :], 0.0)

            wo_sb = []
            for cc in range(2):
                t = pp.tile([128, D], BF16, tag=f"wo{cc}", name=f"wo{cc}")
                nc.sync.dma_start(t[:], woT[cc * 128:(cc + 1) * 128, :])
                wo_sb.append(t)

            with (
                tc.tile_pool(name="xw", bufs=1) as xw,
                tc.tile_pool(name="et", bufs=8) as etp,
                tc.tile_pool(name="nrm", bufs=3) as nrm,
                tc.tile_pool(name="ob", bufs=2) as obp,
                tc.tile_pool(name="ps_sc", bufs=3, space="PSUM") as sc_ps,
                tc.tile_pool(name="ps_av", bufs=3, space="PSUM") as av_ps,
                tc.tile_pool(name="ps_fa", bufs=2, space="PSUM") as fa_ps,
            ):
                xT_sb, wqk_sb, wv_sb = [], [], []
                for m in range(8):
                    t = xw.tile([128, 512], BF16, tag=f"wqk{m}",
                                name=f"wqk{m}")
                    nc.sync.dma_start(t[:], wqkT[m * 128:(m + 1) * 128, :])
                    wqk_sb.append(t)
                    t = xw.tile([128, S], BF16, tag=f"x{m}", name=f"x{m}")
                    nc.sync.dma_start(t[:], xT[m * 128:(m + 1) * 128, :])
                    xT_sb.append(t)
                    t = xw.tile([128, 256], BF16, tag=f"wv{m}", name=f"wv{m}")
                    nc.sync.dma_start(t[:], wvT[m * 128:(m + 1) * 128, :])
                    wv_sb.append(t)

                # ---- QKV / V / out-proj single-bank chunks ----
                def qk_chunk(ft, q4, pool, ptag):
                    scol = slice(q4 * 512, (q4 + 1) * 512)
                    fcol = slice(ft * 128, (ft + 1) * 128)
                    ps = pool.tile([128, 512], F32, tag=ptag, name="qkps")
                    for m in range(8):
                        nc.tensor.matmul(
                            ps[:], wqk_sb[m][:, fcol], xT_sb[m][:, scol],
                            start=(m == 0), stop=(m == 7))
                    dst = (qT if ft < 2 else kpk)[ft % 2]
                    nc.vector.tensor_scalar(
                        out=dst[:, scol], in0=ps[:],
                        scalar1=(0.125 if ft < 2 else 1.0),
                        scalar2=bqk_sb[:, ft:ft + 1],
                        op0=mybir.AluOpType.mult,
                        op1=mybir.AluOpType.add,
                    )
                    if ft >= 2:
                        # scatter the two heads into their padded K tiles
                        t = ft - 2
                        nc.vector.tensor_copy(
                            kT[2 * t][0:64, scol], dst[0:64, scol])
                        nc.vector.tensor_copy(
                            kT[2 * t + 1][64:128, scol], dst[64:128, scol])

                def v_chunk(sb, pool, ptag):
                    scol = slice(sb * 128, (sb + 1) * 128)
                    ps = pool.tile([128, 256], F32, tag=ptag, name="vps")
                    for m in range(8):
                        nc.tensor.matmul(
                            ps[:], xT_sb[m][:, scol], wv_sb[m][:],
                            start=(m == 0), stop=(m == 7))
                    nc.vector.tensor_copy(
                        v_t[:, :, sb, 0:64],
                        ps[:].rearrange("p (s c) -> p s c", s=4))

                def op_block(sb, pool, ptag):
                    ob = obp.tile([128, D], BF16, tag="ob", name="ob")
                    bcol = slice(sb * 128, (sb + 1) * 128)
                    for jh in range(2):
                        jcol = slice(jh * 512, (jh + 1) * 512)
                        ps = pool.tile([128, 512], F32, tag=ptag, name="opps")
                        for cc in range(2):
                            nc.tensor.matmul(
                                ps[:], hoT[cc][:, bcol], wo_sb[cc][:, jcol],
                                start=(cc == 0), stop=(cc == 1))
                        nc.vector.tensor_copy(ob[:, jcol], ps[:])
                    nc.sync.dma_start(out[bcol, :], ob[:])

                # ---- attention ----
                sched = [_slot_schedule(s) for s in range(4)]

                def attn_group(s, ent):
                    g, q0, W, blocks = ent
                    qT_s = qT[s // 2]
                    kT_s = kT[s]
                    nb = len(blocks)
                    av = av_ps.tile([65, W], F32, tag="av", name="av")
                    for bi, (j, lo, hi, tcol, isdiag) in enumerate(blocks):
                        # compute only the live causal window [lo:hi); the
                        # first block always spans [0:W) so the start=True
                        # clear covers every av element
                        sc = sc_ps.tile([128, hi - lo], F32, tag="sc",
                                        name="sc")
                        nc.tensor.matmul(
                            sc[:], kT_s[:, j * 128:(j + 1) * 128],
                            qT_s[:, q0 + lo:q0 + hi])
                        et = etp.tile([128, W], BF16, tag="et", name="et")
                        nc.scalar.activation(
                            et[:, lo:hi], sc[:],
                            mybir.ActivationFunctionType.Exp,
                            bias=btab_sb[:, tcol:tcol + 1], scale=1.0)
                        if isdiag:
                            # zero k>q inside the diagonal 128x128 block
                            nc.gpsimd.affine_select(
                                out=et[:, lo:lo + 128],
                                in_=et[:, lo:lo + 128],
                                compare_op=mybir.AluOpType.is_ge,
                                fill=0.0, base=0,
                                pattern=[[1, 128]],
                                channel_multiplier=-1,
                            )
                        nc.tensor.matmul(
                            av[:, lo:hi], v_t[:, s, j, :], et[:, lo:hi],
                            start=(bi == 0), stop=(bi == nb - 1))
                    # softmax denominator: ones-column sum broadcast via
                    # one-hot matmul, then reciprocal + scale
                    nc.vector.tensor_copy(lr_den[64:65, 0:W], av[64:65, :])
                    bc = sc_ps.tile([128, W], F32, tag="sc", name="bc")
                    nc.tensor.matmul(bc[:], oh_sb[:], lr_den[:, 0:W])
                    binv = nrm.tile([64, W], F32, tag="binv", name="binv")
                    nc.vector.reciprocal_approx_fast(
                        out=binv[:], in_=bc[0:64, :])
                    hoT_s = hoT[s // 2]
                    po = (s % 2) * 64
                    if po == 0:
                        nc.vector.tensor_mul(
                            hoT_s[0:64, q0:q0 + W], av[0:64, :], binv[:])
                    else:
                        # DVE lanes can't shift partitions; bounce via DMA
                        tmp = nrm.tile([64, W], BF16, tag="hotmp",
                                       name="hotmp")
                        nc.vector.tensor_mul(tmp[:], av[0:64, :], binv[:])
                        nc.sync.dma_start(
                            hoT_s[64:128, q0:q0 + W], tmp[:])

                # ---- emission ----
                # startup: all chunks for q half 0, deep bank rotation
                rot = [(fa_ps, "fa"), (av_ps, "av"), (sc_ps, "sc")]
                ri = 0

                def next_pool():
                    nonlocal ri
                    p = rot[ri % len(rot)]
                    ri += 1
                    return p

                for q4 in range(2):
                    for ft in (2, 3, 0, 1):       # k first: extra copy hop
                        p, t = next_pool()
                        qk_chunk(ft, q4, p, t)
                    for sb in (4 * q4, 4 * q4 + 1, 4 * q4 + 2, 4 * q4 + 3):
                        p, t = next_pool()
                        v_chunk(sb, p, t)

                # filler queue consumed between attention groups
                fillers = []
                for q4 in (2, 3):
                    for ft in (2, 3, 0, 1):
                        fillers.append(("qk", ft, q4))
                for sb in range(8, 16):
                    fillers.append(("v", sb))
                frot = [(fa_ps, "fa")]
                fi = 0

                def pop_fillers(n):
                    nonlocal fi
                    for _ in range(n):
                        if not fillers:
                            return
                        kind, *a = fillers.pop(0)
                        p, t = frot[0]
                        fi += 1
                        if kind == "qk":
                            qk_chunk(a[0], a[1], p, t)
                        elif kind == "v":
                            v_chunk(a[0], p, t)
                        else:
                            op_block(a[0], p, t)

                for g in range(4):
                    for s in range(3):
                        attn_group(s, sched[s][g])
                        pop_fillers(2)
                    for i4 in range(4):
                        attn_group(3, sched[3][4 * g + i4])
                    pop_fillers(2)
                    for sb in range(4 * g, 4 * g + 4):
                        fillers.append(("op", sb))
                while fillers:
                    pop_fillers(1)

    nc.compile()
    return nc


def make_core_inputs(c, x, W_packed, b_packed):
    """Host-side shard prep for core c (pure numpy reshuffles)."""
    k, b = c % 4, c // 4
    heads = [12 + k, 8 + k, 4 + k, k]          # slots A..D
    rows = np.concatenate([np.arange(h * 64, (h + 1) * 64) for h in heads])

    xT = np.ascontiguousarray(x[b].T)                       # [D, S]
    wq = W_packed[rows]
    wk = W_packed[D + rows]
    wv = W_packed[2 * D + rows]
    wqkT = np.ascontiguousarray(np.concatenate([wq, wk], 0).T)  # [D, 512]
    wvT = np.ascontiguousarray(wv.T)                        # [D, 256]

    bq = b_packed[rows] / 8.0
    bk = b_packed[D + rows]
    bqk = np.stack([bq[:128], bq[128:], bk[:128], bk[128:]], 1)  # [128, 4]
    bqk = np.ascontiguousarray(bqk, dtype=np.float32)

    btab = np.zeros((128, TABW), np.float32)
    p = np.arange(128, dtype=np.float64)[:, None]
    for s in range(4):
        h = heads[s]
        slope = 2.0 ** (-(h + 1) * 8.0 / H)
        K, off0, tw, to = SLOT_KEEP[s], SLOT_OFF0[s], SLOT_TABW[s], SLOT_TABOFF[s]
        m = np.arange(tw, dtype=np.float64)[None, :]
        btab[:, to:to + tw] = (slope * (p + 128.0 * (m - (K - 1)) - off0)
                               ).astype(np.float32)
    import ml_dtypes
    onehot = np.zeros((128, 128), np.float32)
    onehot[64, 0:64] = 1.0
    return heads, {"xT": xT.astype(ml_dtypes.bfloat16),
                   "wqkT": wqkT.astype(ml_dtypes.bfloat16),
                   "wvT": wvT.astype(ml_dtypes.bfloat16),
                   "bqk": bqk, "btab": btab, "onehot": onehot,
                   "zr": np.zeros((128, 512), np.float32)}


_NC_CACHE = {}


def _get_program():
    if "nc" not in _NC_CACHE:
        _NC_CACHE["nc"] = build_program()
    return _NC_CACHE["nc"]


def kernel(x, W_packed, b_packed, W_out, b_out):
    x = np.asarray(x, np.float32)
    W_packed = np.asarray(W_packed, np.float32)
    b_packed = np.asarray(b_packed, np.float32)
    W_out = np.asarray(W_out, np.float32)
    b_out = np.asarray(b_out, np.float32)

    nc = _get_program()

    in_maps = []
    for c in range(NCORES):
        heads, m = make_core_inputs(c, x, W_packed, b_packed)
        cols = np.concatenate([np.arange(h * 64, (h + 1) * 64) for h in heads])
        import ml_dtypes
        m["woT"] = np.ascontiguousarray(W_out[:, cols].T).astype(
            ml_dtypes.bfloat16)
        in_maps.append(m)

    res = run_bass_kernel_spmd(nc, in_maps, core_ids=list(range(NCORES)))

    # Gather: sum partials per batch; add b_out and the folded v-bias term.
    b_v = b_packed[2 * D:]
    bias_row = (b_out + W_out @ b_v).astype(np.float32)     # [D]
    full = np.empty((B, S, D), np.float32)
    for b in range(B):
        acc = res.results[4 * b]["out"].astype(np.float32).copy()
        for c in range(4 * b + 1, 4 * b + 4):
            acc += res.results[c]["out"]
        full[b] = acc + bias_row
    return full


# revision 16
# speedup vs baseline: 1.4829x; 1.2708x over previous
"""Trainium2 Bass kernel for causal multi-head attention with ALiBi.

Computes, for x:[B,S,D]:
    qkv = x @ W_packed.T + b_packed ; q,k,v = split(qkv)
    heads -> scores = q k^T / sqrt(hd) + alibi_causal_bias
    out = softmax(scores) v -> merge heads -> out @ W_out.T + b_out

Sharding (8 cores): core c handles batch c//4 and heads {k, k+4, k+8, k+12}
(k = c%4), one head per "slot". Host sums the 4 out-projection partials per
batch and adds b_out + W_out @ b_v.

Perf structure: every matmul runs the PE array in the same (128,128)
configuration - reconfiguring the array's tiling mode drains it (~110ns,
measured), and a per-head schedule would pay that on every score/AV matmul
pair.  Scores contract over hd=64, so each slot's K tile is stored
zero-padded to 128 partitions (data on its own half, zeros on the other);
the packed Q tile streams both heads and the zero half of K kills the
other head's contribution.  Softmax row sums ride a ones-column appended
to v; 1/den is broadcast across partitions by a one-hot f32r matmul.
QKV and out-proj are emitted as single-bank chunks woven between
attention groups as PE filler under the scalar engine's exp stream.

ALiBi sparsity: head h attends effectively only ~35/slope_h positions
back. Slots keep only the causal k-blocks within that window.  Softmax
without row-max: exp is recentred per q-group by a per-column constant
that cancels in normalization (rides the Exp bias table).
"""

import os
import sys

import numpy as np

for _p in ("/opt/trn_rl_repo",):
    if os.path.isdir(_p) and _p not in sys.path:
        sys.path.append(_p)

import concourse.bacc as bacc
import concourse.bass as bass
import concourse.tile as tile
from concourse import mybir
from concourse.bass_utils import run_bass_kernel_spmd

B, S, D, H, HD = 2, 2048, 1024, 16, 64
NBLK = S // 128          # 16 k/q blocks
NCORES = 8

F32 = mybir.dt.float32
F32R = mybir.dt.float32r
BF16 = mybir.dt.bfloat16

SLOT_KEEP = (17, 17, 6, 3)
SLOT_W = (512, 512, 512, 128)
SLOT_OFF0 = (128, 128, 128, 64)
SLOT_TABW = tuple(k + 3 if w == 512 else k for k, w in zip(SLOT_KEEP, SLOT_W))
SLOT_TABOFF = tuple(int(np.cumsum((0,) + SLOT_TABW)[i]) for i in range(4))
TABW = int(sum(SLOT_TABW))  # 52


def _slot_schedule(s):
    """Yield (g, q0, W, [(j, lo, hi, tabcol, isdiag), ...]) per q-group."""
    K, W, _ = SLOT_KEEP[s], SLOT_W[s], SLOT_OFF0[s]
    out = []
    if W == 512:
        for g in range(S // 512):
            jlo = max(0, 4 * g + 3 - (K - 1))
            blocks = []
            for j in range(jlo, 4 * g + 4):
                lo = max(0, (j - 4 * g) * 128)
                hi = min(512, (j - 4 * g + K) * 128)
                m = j - 4 * g + (K - 1)
                blocks.append((j, lo, hi, SLOT_TABOFF[s] + m, j >= 4 * g))
            out.append((g, g * 512, 512, blocks))
    else:
        for i in range(NBLK):
            blocks = []
            for j in range(max(0, i - (K - 1)), i + 1):
                m = j - i + (K - 1)
                blocks.append((j, 0, 128, SLOT_TABOFF[s] + m, j == i))
            out.append((i, i * 128, 128, blocks))
    return out


def build_program():
    nc = bacc.Bacc("TRN2", target_bir_lowering=False, debug=False,
                   num_devices=NCORES)

    xT = nc.dram_tensor("xT", [D, S], BF16, kind="ExternalInput")
    wqkT = nc.dram_tensor("wqkT", [D, 512], BF16, kind="ExternalInput")
    wvT = nc.dram_tensor("wvT", [D, 256], BF16, kind="ExternalInput")
    woT = nc.dram_tensor("woT", [256, D], BF16, kind="ExternalInput")
    bqk = nc.dram_tensor("bqk", [128, 4], F32, kind="ExternalInput")
    btab = nc.dram_tensor("btab", [128, TABW], F32, kind="ExternalInput")
    onehot = nc.dram_tensor("onehot", [128, 128], BF16, kind="ExternalInput")
    zr = nc.dram_tensor("zr", [128, 512], BF16, kind="ExternalInput")
    out = nc.dram_tensor("out", [S, D], BF16, kind="ExternalOutput")

    with tile.TileContext(nc) as tc:
        with tc.tile_pool(name="persist", bufs=1) as pp:
            # q tiles packed (2 heads per tile); k tiles zero-padded per slot
            qT = [pp.tile([128, S], BF16, tag=f"qT{t}", name=f"qT{t}")
                  for t in range(2)]
            kpk = [pp.tile([128, S], BF16, tag=f"kpk{t}", name=f"kpk{t}")
                   for t in range(2)]
            kT = [pp.tile([128, S], BF16, tag=f"kT{s}", name=f"kT{s}")
                  for s in range(4)]
            v_t = pp.tile([128, 4, NBLK, 65], BF16, tag="v", name="v")
            hoT = [pp.tile([128, S], BF16, tag=f"hoT{t}", name=f"hoT{t}")
                   for t in range(2)]
            btab_sb = pp.tile([128, TABW], F32, tag="btab", name="btab")
            bqk_sb = pp.tile([128, 4], F32, tag="bqk", name="bqk")
            oh_sb = pp.tile([128, 128], BF16, tag="oh", name="oh")
            # den staging row (row 64); other rows must stay zero (they are
            # contraction rows of the one-hot matmul; garbage would poison)
            lr_den = pp.tile([128, 512], BF16, tag="lrd", name="lrd")

            nc.sync.dma_start(btab_sb[:], btab[:])
            nc.sync.dma_start(bqk_sb[:], bqk[:])
            nc.sync.dma_start(oh_sb[:], onehot[:])
            nc.sync.dma_start(lr_den[:], zr[:])
            nc.gpsimd.memset(v_t[:, :, :, 64:65], 1.0)
            # zero the pad half of each slot's K tile (once)
            for s in range(4):
                po = (s % 2) * 64
                nc.gpsimd.memset(kT[s][64 - po:128 - po, :], 0.0)

            wo_sb = []
            for cc in range(2):
                t = pp.tile([128, D], BF16, tag=f"wo{cc}", name=f"wo{cc}")
                nc.sync.dma_start(t[:], woT[cc * 128:(cc + 1) * 128, :])
                wo_sb.append(t)

            with (
                tc.tile_pool(name="xw", bufs=1) as xw,
                tc.tile_pool(name="et", bufs=12) as etp,
                tc.tile_pool(name="nrm", bufs=3) as nrm,
                tc.tile_pool(name="ob", bufs=2) as obp,
                tc.tile_pool(name="ps_sc", bufs=3, space="PSUM") as sc_ps,
                tc.tile_pool(name="ps_av", bufs=3, space="PSUM") as av_ps,
                tc.tile_pool(name="ps_fa", bufs=2, space="PSUM") as fa_ps,
            ):
                xT_sb, wqk_sb, wv_sb = [], [], []
                for m in range(8):
                    t = xw.tile([128, 512], BF16, tag=f"wqk{m}",
                                name=f"wqk{m}")
                    nc.sync.dma_start(t[:], wqkT[m * 128:(m + 1) * 128, :])
                    wqk_sb.append(t)
                    t = xw.tile([128, S], BF16, tag=f"x{m}", name=f"x{m}")
                    nc.sync.dma_start(t[:], xT[m * 128:(m + 1) * 128, :])
                    xT_sb.append(t)
                    t = xw.tile([128, 256], BF16, tag=f"wv{m}", name=f"wv{m}")
                    nc.sync.dma_start(t[:], wvT[m * 128:(m + 1) * 128, :])
                    wv_sb.append(t)

                # ---- QKV / V / out-proj single-bank chunks ----
                def qk_chunk(ft, q4, pool, ptag):
                    scol = slice(q4 * 512, (q4 + 1) * 512)
                    fcol = slice(ft * 128, (ft + 1) * 128)
                    ps = pool.tile([128, 512], F32, tag=ptag, name="qkps")
                    for m in range(8):
                        nc.tensor.matmul(
                            ps[:], wqk_sb[m][:, fcol], xT_sb[m][:, scol],
                            start=(m == 0), stop=(m == 7))
                    dst = (qT if ft < 2 else kpk)[ft % 2]
                    nc.vector.tensor_scalar(
                        out=dst[:, scol], in0=ps[:],
                        scalar1=(0.125 if ft < 2 else 1.0),
                        scalar2=bqk_sb[:, ft:ft + 1],
                        op0=mybir.AluOpType.mult,
                        op1=mybir.AluOpType.add,
                    )
                    if ft >= 2:
                        # scatter the two heads into their padded K tiles
                        t = ft - 2
                        nc.vector.tensor_copy(
                            kT[2 * t][0:64, scol], dst[0:64, scol])
                        nc.vector.tensor_copy(
                            kT[2 * t + 1][64:128, scol], dst[64:128, scol])

                def v_chunk(sb, pool, ptag):
                    scol = slice(sb * 128, (sb + 1) * 128)
                    ps = pool.tile([128, 256], F32, tag=ptag, name="vps")
                    for m in range(8):
                        nc.tensor.matmul(
                            ps[:], xT_sb[m][:, scol], wv_sb[m][:],
                            start=(m == 0), stop=(m == 7))
                    nc.vector.tensor_copy(
                        v_t[:, :, sb, 0:64],
                        ps[:].rearrange("p (s c) -> p s c", s=4))

                def op_block(sb, pool, ptag):
                    ob = obp.tile([128, D], BF16, tag="ob", name="ob")
                    bcol = slice(sb * 128, (sb + 1) * 128)
                    for jh in range(2):
                        jcol = slice(jh * 512, (jh + 1) * 512)
                        ps = pool.tile([128, 512], F32, tag=ptag, name="opps")
                        for cc in range(2):
                            nc.tensor.matmul(
                                ps[:], hoT[cc][:, bcol], wo_sb[cc][:, jcol],
                                start=(cc == 0), stop=(cc == 1))
                        nc.vector.tensor_copy(ob[:, jcol], ps[:])
                    nc.sync.dma_start(out[bcol, :], ob[:])

                # ---- attention ----
                sched = [_slot_schedule(s) for s in range(4)]

                def attn_group(s, ent):
                    g, q0, W, blocks = ent
                    qT_s = qT[s // 2]
                    kT_s = kT[s]
                    # diag blocks first: their affine_select latency hides
                    # under the other blocks' exps instead of the group tail
                    blocks = sorted(blocks, key=lambda b: not b[4])
                    nb = len(blocks)
                    av = av_ps.tile([65, W], F32, tag="av", name="av")
                    ets = []
                    for j, lo, hi, tcol, isdiag in blocks:
                        sc = sc_ps.tile([128, hi - lo], F32, tag="sc",
                                        name="sc")
                        nc.tensor.matmul(
                            sc[:], kT_s[:, j * 128:(j + 1) * 128],
                            qT_s[:, q0 + lo:q0 + hi])
                        et = etp.tile([128, W], BF16, tag="et", name="et")
                        nc.scalar.activation(
                            et[:, lo:hi], sc[:],
                            mybir.ActivationFunctionType.Exp,
                            bias=btab_sb[:, tcol:tcol + 1], scale=1.0)
                        if isdiag:
                            # zero k>q inside the diagonal 128x128 block
                            nc.gpsimd.affine_select(
                                out=et[:, lo:lo + 128],
                                in_=et[:, lo:lo + 128],
                                compare_op=mybir.AluOpType.is_ge,
                                fill=0.0, base=0,
                                pattern=[[1, 128]],
                                channel_multiplier=-1,
                            )
                        ets.append(et)
                    avorder = sorted(range(nb), key=lambda i: blocks[i][0])
                    for oi, bi in enumerate(avorder):
                        j, lo, hi, tcol, isdiag = blocks[bi]
                        nc.tensor.matmul(
                            av[:, lo:hi], v_t[:, s, j, :], ets[bi][:, lo:hi],
                            start=(oi == 0), stop=(oi == nb - 1))
                    # softmax denominator: ones-column sum broadcast via
                    # one-hot matmul, then reciprocal + scale
                    nc.vector.tensor_copy(lr_den[64:65, 0:W], av[64:65, :])
                    bc = av_ps.tile([128, W], F32, tag="av", name="bc")
                    nc.tensor.matmul(bc[:], oh_sb[:], lr_den[:, 0:W])
                    binv = nrm.tile([64, W], F32, tag="binv", name="binv")
                    nc.vector.reciprocal_approx_fast(
                        out=binv[:], in_=bc[0:64, :])
                    hoT_s = hoT[s // 2]
                    po = (s % 2) * 64
                    if po == 0:
                        nc.vector.tensor_mul(
                            hoT_s[0:64, q0:q0 + W], av[0:64, :], binv[:])
                    else:
                        # DVE lanes can't shift partitions; bounce via DMA
                        tmp = nrm.tile([64, W], BF16, tag="hotmp",
                                       name="hotmp")
                        nc.vector.tensor_mul(tmp[:], av[0:64, :], binv[:])
                        nc.sync.dma_start(
                            hoT_s[64:128, q0:q0 + W], tmp[:])

                # ---- emission ----
                # startup: all chunks for q half 0, deep bank rotation
                rot = [(fa_ps, "fa"), (av_ps, "av"), (sc_ps, "sc")]
                ri = 0

                def next_pool():
                    nonlocal ri
                    p = rot[ri % len(rot)]
                    ri += 1
                    return p


                # filler queue consumed between attention groups
                fillers = []
                for q4 in (2, 3):
                    for ft in (2, 3, 0, 1):
                        fillers.append(("qk", ft, q4))
                for sb in range(8, 16):
                    fillers.append(("v", sb))
                frot = [(fa_ps, "fa")]
                fi = 0

                def pop_fillers(n):
                    nonlocal fi
                    for _ in range(n):
                        if not fillers:
                            return
                        kind, *a = fillers.pop(0)
                        p, t = frot[0]
                        fi += 1
                        if kind == "qk":
                            qk_chunk(a[0], a[1], p, t)
                        elif kind == "v":
                            v_chunk(a[0], p, t)
                        else:
                            op_block(a[0], p, t)

                for q4 in range(2):
                    for ft in (2, 3, 0, 1):       # k first: extra copy hop
                        p, t = next_pool()
                        qk_chunk(ft, q4, p, t)
                    for sb in (4 * q4, 4 * q4 + 1, 4 * q4 + 2, 4 * q4 + 3):
                        p, t = next_pool()
                        v_chunk(sb, p, t)
                    for s in range(3):
                        attn_group(s, sched[s][q4])
                        pop_fillers(2)
                    for i4 in range(4):
                        attn_group(3, sched[3][4 * q4 + i4])
                    pop_fillers(2)
                    fillers.extend(
                        ("op", sb) for sb in range(4 * q4, 4 * q4 + 4))

                for g in range(2, 4):
                    for s in range(3):
                        attn_group(s, sched[s][g])
                        pop_fillers(2)
                    for i4 in range(4):
                        attn_group(3, sched[3][4 * g + i4])
                    pop_fillers(2)
                    for sb in range(4 * g, 4 * g + 4):
                        fillers.append(("op", sb))
                while fillers:
                    pop_fillers(1)

    nc.compile()
    return nc


def make_core_inputs(c, x, W_packed, b_packed):
    """Host-side shard prep for core c (pure numpy reshuffles)."""
    k, b = c % 4, c // 4
    heads = [12 + k, 8 + k, 4 + k, k]          # slots A..D
    rows = np.concatenate([np.arange(h * 64, (h + 1) * 64) for h in heads])

    xT = np.ascontiguousarray(x[b].T)                       # [D, S]
    wq = W_packed[rows]
    wk = W_packed[D + rows]
    wv = W_packed[2 * D + rows]
    wqkT = np.ascontiguousarray(np.concatenate([wq, wk], 0).T)  # [D, 512]
    wvT = np.ascontiguousarray(wv.T)                        # [D, 256]

    bq = b_packed[rows] / 8.0
    bk = b_packed[D + rows]
    bqk = np.stack([bq[:128], bq[128:], bk[:128], bk[128:]], 1)  # [128, 4]
    bqk = np.ascontiguousarray(bqk, dtype=np.float32)

    btab = np.zeros((128, TABW), np.float32)
    p = np.arange(128, dtype=np.float64)[:, None]
    for s in range(4):
        h = heads[s]
        slope = 2.0 ** (-(h + 1) * 8.0 / H)
        K, off0, tw, to = SLOT_KEEP[s], SLOT_OFF0[s], SLOT_TABW[s], SLOT_TABOFF[s]
        m = np.arange(tw, dtype=np.float64)[None, :]
        btab[:, to:to + tw] = (slope * (p + 128.0 * (m - (K - 1)) - off0)
                               ).astype(np.float32)
    import ml_dtypes
    onehot = np.zeros((128, 128), np.float32)
    onehot[64, 0:64] = 1.0
    onehot = onehot.astype(ml_dtypes.bfloat16)
    return heads, {"xT": xT.astype(ml_dtypes.bfloat16),
                   "wqkT": wqkT.astype(ml_dtypes.bfloat16),
                   "wvT": wvT.astype(ml_dtypes.bfloat16),
                   "bqk": bqk, "btab": btab, "onehot": onehot,
                   "zr": np.zeros((128, 512), np.float32).astype(
                       ml_dtypes.bfloat16)}


_NC_CACHE = {}


def _get_program():
    if "nc" not in _NC_CACHE:
        _NC_CACHE["nc"] = build_program()
    return _NC_CACHE["nc"]


def kernel(x, W_packed, b_packed, W_out, b_out):
    x = np.asarray(x, np.float32)
    W_packed = np.asarray(W_packed, np.float32)
    b_packed = np.asarray(b_packed, np.float32)
    W_out = np.asarray(W_out, np.float32)
    b_out = np.asarray(b_out, np.float32)

    nc = _get_program()

    in_maps = []
    for c in range(NCORES):
        heads, m = make_core_inputs(c, x, W_packed, b_packed)
        cols = np.concatenate([np.arange(h * 64, (h + 1) * 64) for h in heads])
        import ml_dtypes
        m["woT"] = np.ascontiguousarray(W_out[:, cols].T).astype(
            ml_dtypes.bfloat16)
        in_maps.append(m)

    res = run_bass_kernel_spmd(nc, in_maps, core_ids=list(range(NCORES)))

    # Gather: sum partials per batch; add b_out and the folded v-bias term.
    b_v = b_packed[2 * D:]
    bias_row = (b_out + W_out @ b_v).astype(np.float32)     # [D]
    full = np.empty((B, S, D), np.float32)
    for b in range(B):
        acc = res.results[4 * b]["out"].astype(np.float32).copy()
        for c in range(4 * b + 1, 4 * b + 4):
            acc += res.results[c]["out"]
        full[b] = acc + bias_row
    return full
